# revision 19
# baseline (speedup 1.0000x reference)
"""Trainium2 Bass kernel for a dense transformer block (B=4, N=2048, C=768,
H=12, D=64, HID=3072), sharded over 8 NeuronCores.

Sharding: token-split, no collectives. Core s handles batch b = s//2,
sequence half = s%2 (1024 tokens). Each core receives its batch element's
full 2048-token x (rolled so its own tokens are rows 0..1023), computes
K/V over all 2048 tokens (redundantly with its pair core), and produces
the output for its own 1024 tokens. Host gathers/transposes.

Layout: activations are kept feature-major ("X^T", [C, tokens]) so every
linear layer is a natural PE matmul (weights pre-transposed on host).
Attention computes S^T = (K^T_h).T-tiles @ Q^T_h with softmax along the
partition (key) axis: no max subtraction (logits are O(1) here), sums via
an appended ones-column on V, normalization via PE ones-broadcast.
"""

import numpy as np

import concourse.bass as bass
import concourse.mybir as mybir
import concourse.tile as tile
from concourse import bacc
from concourse.bass_utils import run_bass_kernel_spmd
from concourse.masks import make_identity

F32 = mybir.dt.float32
F32R = mybir.dt.float32r
BF16 = mybir.dt.bfloat16
AF = mybir.ActivationFunctionType
ALU = mybir.AluOpType

B, N, C = 4, 2048, 768
H, D = 12, 64
HID = 3072
EPS = 1e-5
NCORES = 8
NO = 1024  # tokens owned per core
NKV = 2048  # key/value tokens per core
CT = C // 128  # 6 feature tiles
HT = HID // 128  # 24 hidden tiles
KT = NKV // 128  # 16 kv token tiles
QCH = NO // 512  # 2 query chunks of 512
ISCALE = 1.0 / np.sqrt(D)

LAST_RESULTS = None
_NC_CACHE = None


def build_program(repeats=1):
    nc = bacc.Bacc(trn_type="TRN2", target_bir_lowering=False, num_devices=NCORES)

    xb = nc.dram_tensor("xb", [NKV, C], F32, kind="ExternalInput").ap()
    wqkvT = nc.dram_tensor("wqkvT", [C, 3 * C], F32R, kind="ExternalInput").ap()
    wprojT = nc.dram_tensor("wprojT", [C, C], F32R, kind="ExternalInput").ap()
    wfc1T = nc.dram_tensor("wfc1T", [C, HID], F32R, kind="ExternalInput").ap()
    wfc2T = nc.dram_tensor("wfc2T", [HID, C], F32R, kind="ExternalInput").ap()
    pb = nc.dram_tensor("pb", [C], F32, kind="ExternalInput").ap()
    f1b = nc.dram_tensor("f1b", [HID], F32, kind="ExternalInput").ap()
    f2b = nc.dram_tensor("f2b", [C], F32, kind="ExternalInput").ap()
    g1 = nc.dram_tensor("g1", [C], F32, kind="ExternalInput").ap()
    b1 = nc.dram_tensor("b1", [C], F32, kind="ExternalInput").ap()
    g2 = nc.dram_tensor("g2", [C], F32, kind="ExternalInput").ap()
    b2 = nc.dram_tensor("b2", [C], F32, kind="ExternalInput").ap()
    outT = nc.dram_tensor("outT", [C, NO], F32, kind="ExternalOutput").ap()

    with tile.TileContext(nc) as tc:
        for _ in range(repeats):
            emit(nc, tc, xb, wqkvT, wprojT, wfc1T, wfc2T, pb, f1b, f2b,
                 g1, b1, g2, b2, outT)
    nc.compile()
    return nc


def emit(nc, tc, xb, wqkvT, wprojT, wfc1T, wfc2T, pb, f1b, f2b,
         g1, b1, g2, b2, outT):
    dma = nc.sync.dma_start

    with (
        tc.tile_pool(name="consts", bufs=1) as consts,
        tc.tile_pool(name="x2T", bufs=1) as x2T_pool,
    ):
        # ---- constants ----
        ident = consts.tile([128, 128], F32, tag="ident")
        make_identity(nc, ident)
        ones_f32 = consts.tile([128, 2], F32, tag="ones_f32")
        nc.vector.memset(ones_f32, 1.0)
        ones_row = consts.tile([1, 128], F32R, tag="ones_row")
        nc.scalar.activation(out=ones_row, in_=ones_f32[0:1, 0:1].broadcast_to(
            [1, 128]), func=AF.Copy)
        ones_col = consts.tile([128, 1], F32R, tag="ones_col")
        nc.scalar.activation(out=ones_col, in_=ones_f32[:, 0:1], func=AF.Copy)
        eps_t = consts.tile([128, 1], F32, tag="eps")
        nc.vector.memset(eps_t, EPS)
        # per-feature vectors as [128, CT] (col ct = features ct*128..)
        g1_s = consts.tile([128, CT], F32, tag="g1")
        dma(out=g1_s, in_=g1.rearrange("(ct p) -> p ct", p=128))
        b1_s = consts.tile([128, CT], F32, tag="b1")
        dma(out=b1_s, in_=b1.rearrange("(ct p) -> p ct", p=128))
        g2_s = consts.tile([128, CT], F32, tag="g2")
        dma(out=g2_s, in_=g2.rearrange("(ct p) -> p ct", p=128))
        b2_s = consts.tile([128, CT], F32, tag="b2")
        dma(out=b2_s, in_=b2.rearrange("(ct p) -> p ct", p=128))
        pb_s = consts.tile([128, CT], F32, tag="pb")
        dma(out=pb_s, in_=pb.rearrange("(ct p) -> p ct", p=128))
        f2b_s = consts.tile([128, CT], F32, tag="f2b")
        dma(out=f2b_s, in_=f2b.rearrange("(ct p) -> p ct", p=128))
        f1b_s = consts.tile([128, HT], F32, tag="f1b")
        dma(out=f1b_s, in_=f1b.rearrange("(ht p) -> p ht", p=128))

        x2T = [x2T_pool.tile([128, NO], F32R, tag=f"x2T{ct}", name=f"x2T{ct}") for ct in range(CT)]

        with (
            tc.tile_pool(name="qT", bufs=1) as qT_pool,
            tc.tile_pool(name="kT", bufs=1) as kT_pool,
            tc.tile_pool(name="vA", bufs=1) as vA_pool,
        ):
            qT = [qT_pool.tile([128, NO], BF16, tag=f"qT{ct}", name=f"qT{ct}") for ct in range(CT)]
            kT = [kT_pool.tile([128, NKV], BF16, tag=f"kT{ct}", name=f"kT{ct}") for ct in range(CT)]
            # V (token-major) with a ones column appended per head:
            # [kv_tokens, head, D+1]
            vA = [vA_pool.tile([128, H, D + 1], BF16, tag=f"vA{nt}", name=f"vA{nt}")
                  for nt in range(KT)]

            # ============ Phase 1+2: LN1, transposes, QKV ============
            with tc.tile_pool(name="hkvT", bufs=1) as hkvT_pool:
                hkvT = [hkvT_pool.tile([128, NKV], F32R, tag=f"hkvT{ct}", name=f"hkvT{ct}")
                        for ct in range(CT)]
                with (
                    tc.tile_pool(name="ln1_work", bufs=2) as lw,
                    tc.tile_pool(name="ln1_stat", bufs=6) as lstat,
                    tc.tile_pool(name="tr_psum", bufs=2, space="PSUM") as trp,
                ):
                    for g in range(KT // 4):  # groups of 4 token tiles
                        xcs = []
                        for j in range(4):
                            nt = 4 * g + j
                            xt = lw.tile([128, C], F32, tag=f"xt{j}",
                                         name=f"xt{j}")
                            dma(out=xt, in_=xb[nt * 128:(nt + 1) * 128, :])
                            # mean/var over C via bn_stats on 3 subgroups
                            st = lstat.tile([128, 3, 6], F32, tag="st")
                            xg = xt.rearrange("p (s d) -> p s d", s=3)
                            for s in range(3):
                                nc.vector.bn_stats(out=st[:, s], in_=xg[:, s])
                            mv = lstat.tile([128, 2], F32, tag="mv")
                            nc.vector.bn_aggr(out=mv, in_=st)
                            rstd = lstat.tile([128, 1], F32, tag="rstd")
                            nc.scalar.activation(out=rstd, in_=mv[:, 1:2],
                                                 func=AF.Sqrt,
                                                 bias=eps_t, scale=1.0)
                            nc.vector.reciprocal(out=rstd, in_=rstd)
                            nmr = lstat.tile([128, 1], F32, tag="nmr")
                            nc.vector.tensor_scalar(out=nmr, in0=mv[:, 0:1],
                                                    scalar1=-1.0, scalar2=rstd,
                                                    op0=ALU.mult, op1=ALU.mult)
                            xc = lw.tile([128, C], F32, tag=f"xc{j}",
                                         name=f"xc{j}")
                            nc.scalar.activation(out=xc, in_=xt,
                                                 func=AF.Identity,
                                                 scale=rstd, bias=nmr)
                            xcs.append(xc)
                        for ct in range(CT):
                            ps = trp.tile([128, 512], F32, tag="tr")
                            for j in range(4):
                                nc.tensor.transpose(
                                    ps[:, j * 128:(j + 1) * 128],
                                    xcs[j][:, ct * 128:(ct + 1) * 128], ident)
                            nc.scalar.activation(
                                out=hkvT[ct][:, g * 512:(g + 1) * 512],
                                in_=ps, func=AF.Identity,
                                scale=g1_s[:, ct:ct + 1], bias=b1_s[:, ct:ct + 1])

                # QKV projections
                with (
                    tc.tile_pool(name="wqkv", bufs=1) as wq_pool,
                    tc.tile_pool(name="mmq_psum", bufs=2, space="PSUM") as mmq,
                    tc.tile_pool(name="mmv_psum", bufs=2, space="PSUM") as mmvA,
                    tc.tile_pool(name="mmv2_psum", bufs=2, space="PSUM") as mmvB,
                ):
                    wq = wq_pool.tile([128, CT, 3 * C], F32R, tag="wqkv")
                    dma(out=wq, in_=wqkvT.rearrange("(ct p) f -> p ct f", p=128))

                    # Q^T, K^T (feature-major, bf16)
                    for ft in range(CT):
                        for ch in range(QCH):
                            ps = mmq.tile([128, 512], F32, tag="mmq")
                            for ct in range(CT):
                                nc.tensor.matmul(
                                    ps,
                                    wq[:, ct, ft * 128:(ft + 1) * 128],
                                    hkvT[ct][:, ch * 512:(ch + 1) * 512],
                                    start=(ct == 0), stop=(ct == CT - 1))
                            nc.vector.tensor_copy(
                                out=qT[ft][:, ch * 512:(ch + 1) * 512], in_=ps)
                        for ch in range(NKV // 512):
                            ps = mmq.tile([128, 512], F32, tag="mmq")
                            for ct in range(CT):
                                nc.tensor.matmul(
                                    ps,
                                    wq[:, ct, C + ft * 128:C + (ft + 1) * 128],
                                    hkvT[ct][:, ch * 512:(ch + 1) * 512],
                                    start=(ct == 0), stop=(ct == CT - 1))
                            nc.vector.tensor_copy(
                                out=kT[ft][:, ch * 512:(ch + 1) * 512], in_=ps)
                    # V (token-major, bf16, ones column)
                    for nt in range(KT):
                        psA = mmvA.tile([128, 512], F32, tag="mmvA")
                        psB = mmvB.tile([128, 256], F32, tag="mmvB")
                        for ct in range(CT):
                            hk = hkvT[ct][:, nt * 128:(nt + 1) * 128]
                            nc.tensor.matmul(psA, hk, wq[:, ct, 2 * C:2 * C + 512],
                                             start=(ct == 0), stop=(ct == CT - 1))
                            nc.tensor.matmul(psB, hk, wq[:, ct, 2 * C + 512:3 * C],
                                             start=(ct == 0), stop=(ct == CT - 1))
                        nc.vector.tensor_copy(
                            out=vA[nt][:, 0:8, 0:D],
                            in_=psA.rearrange("p (h d) -> p h d", d=D))
                        nc.vector.tensor_copy(
                            out=vA[nt][:, 8:12, 0:D],
                            in_=psB.rearrange("p (h d) -> p h d", d=D))
                        nc.vector.memset(vA[nt][:, :, D:D + 1], 1.0)

            # ============ Phase 3: attention ============
            with tc.tile_pool(name="oT", bufs=1) as oT_pool:
                oT = [oT_pool.tile([128, NO], F32R, tag=f"oT{ct}", name=f"oT{ct}")
                      for ct in range(CT)]
                with (
                    tc.tile_pool(name="p_sb", bufs=4) as p_sb,
                    tc.tile_pool(name="attn_sm", bufs=4) as asm,
                    tc.tile_pool(name="s_psum", bufs=4, space="PSUM") as sps,
                    tc.tile_pool(name="o_psum", bufs=2, space="PSUM") as ops,
                    tc.tile_pool(name="b_psum", bufs=2, space="PSUM") as bps,
                ):
                    for hp in range(CT):  # head pair: 2hp (rows 0:64), 2hp+1
                        for ch in range(QCH):
                            qs = (slice(0, 64), slice(64, 128))
                            qch = slice(ch * 512, (ch + 1) * 512)
                            po = [ops.tile([D + 1, 512], F32, tag="po", name="po")
                                  for _ in range(2)]
                            for nt in range(KT):
                                for i in range(2):
                                    ps = sps.tile([128, 512], F32, tag="ps")
                                    nc.tensor.matmul(
                                        ps,
                                        kT[hp][qs[i], nt * 128:(nt + 1) * 128],
                                        qT[hp][qs[i], qch],
                                        start=True, stop=True,
                                        tile_position=(64 * i, 0))
                                    pt = p_sb.tile([128, 512], BF16, tag="pt")
                                    nc.scalar.activation(out=pt, in_=ps,
                                                         func=AF.Exp,
                                                         scale=ISCALE)
                                    nc.tensor.matmul(
                                        po[i], vA[nt][:, 2 * hp + i, :], pt,
                                        start=(nt == 0), stop=(nt == KT - 1),
                                        skip_group_check=True)
                            for i in range(2):
                                rec = asm.tile([1, 512], F32R, tag="rec")
                                with nc.allow_low_precision(
                                        reason="denominator rounded to f32r"):
                                    nc.vector.reciprocal(
                                        out=rec, in_=po[i][D:D + 1, :])
                                pb_ = bps.tile([D, 512], F32, tag="pbc")
                                nc.tensor.matmul(pb_, ones_row[0:1, 0:D],
                                                 rec, start=True, stop=True)
                                bcs = asm.tile([D, 512], F32, tag="bcs")
                                nc.vector.tensor_copy(out=bcs, in_=pb_)
                                nc.vector.tensor_mul(
                                    oT[hp][qs[i], qch], po[i][0:D, :], bcs)

                # ============ Phase 4: proj + residual ============
                with (
                    tc.tile_pool(name="wproj", bufs=1) as wp_pool,
                    tc.tile_pool(name="xT_work", bufs=2) as xTw,
                    tc.tile_pool(name="p_psum", bufs=2, space="PSUM") as pps,
                    tc.tile_pool(name="tr2_psum", bufs=2, space="PSUM") as tr2,
                ):
                    wp = wp_pool.tile([128, CT, C], F32R, tag="wproj")
                    dma(out=wp, in_=wprojT.rearrange("(ct p) f -> p ct f", p=128))
                    for ch in range(QCH):
                        # reload own x tokens (contiguous) and transpose on PE
                        xts = []
                        for j in range(4):
                            r0 = ch * 512 + j * 128
                            xt = xTw.tile([128, C], F32, tag=f"xr{j}",
                                          name=f"xr{j}")
                            dma(out=xt, in_=xb[r0:r0 + 128, :])
                            xts.append(xt)
                        xoTc = []
                        for ct in range(CT):
                            pst = tr2.tile([128, 512], F32, tag="tr2")
                            for j in range(4):
                                nc.tensor.transpose(
                                    pst[:, j * 128:(j + 1) * 128],
                                    xts[j][:, ct * 128:(ct + 1) * 128], ident)
                            xo = xTw.tile([128, 512], F32, tag=f"xoT{ct}",
                                          name=f"xoT{ct}")
                            nc.vector.tensor_copy(out=xo, in_=pst)
                            xoTc.append(xo)
                        for ft in range(CT):
                            ps = pps.tile([128, 512], F32, tag="pp")
                            for ct in range(CT):
                                nc.tensor.matmul(
                                    ps, wp[:, ct, ft * 128:(ft + 1) * 128],
                                    oT[ct][:, ch * 512:(ch + 1) * 512],
                                    start=(ct == 0), stop=(ct == CT - 1))
                            # x2 = (proj_psum + proj_b) + x
                            nc.vector.scalar_tensor_tensor(
                                out=x2T[ft][:, ch * 512:(ch + 1) * 512],
                                in0=ps, scalar=pb_s[:, ft:ft + 1], in1=xoTc[ft],
                                op0=ALU.add, op1=ALU.add)

        # ============ Phase 5+6: LN2 + MLP (per 512-token chunk) ============
        with (
            tc.tile_pool(name="ln2_sm", bufs=1) as l2s,
            tc.tile_pool(name="h2T", bufs=1) as h2_pool,
            tc.tile_pool(name="sq_work", bufs=2) as sqw,
            tc.tile_pool(name="st_psum", bufs=1, space="PSUM") as stp,
            tc.tile_pool(name="bc_psum", bufs=1, space="PSUM") as bcp,
        ):
            h2T = [[h2_pool.tile([128, 512], F32R, tag=f"h2T{ct}_{ch}", name=f"h2T{ct}_{ch}")
                    for ct in range(CT)] for ch in range(QCH)]
            for ch in range(QCH):
                cs = slice(ch * 512, (ch + 1) * 512)
                psum = stp.tile([1, 512], F32, tag="s1")
                pssq = stp.tile([1, 512], F32, tag="s2")
                for ct in range(CT):
                    sq = sqw.tile([128, 512], F32R, tag="sq")
                    nc.vector.tensor_mul(sq, x2T[ct][:, cs], x2T[ct][:, cs])
                    nc.tensor.matmul(psum, ones_col, x2T[ct][:, cs],
                                     start=(ct == 0), stop=(ct == CT - 1),
                                     skip_group_check=True)
                    nc.tensor.matmul(pssq, ones_col, sq,
                                     start=(ct == 0), stop=(ct == CT - 1),
                                     skip_group_check=True)
                mu = l2s.tile([1, 512], F32, tag="mu")
                nc.scalar.mul(mu, psum, 1.0 / C)
                msq = l2s.tile([1, 512], F32, tag="msq")
                nc.scalar.mul(msq, pssq, 1.0 / C)
                mu2 = l2s.tile([1, 512], F32, tag="mu2")
                nc.vector.tensor_mul(mu2, mu, mu)
                var = l2s.tile([1, 512], F32, tag="var")
                nc.vector.tensor_sub(var, msq, mu2)
                sd = l2s.tile([1, 512], F32, tag="sd2")
                nc.scalar.activation(out=sd, in_=var, func=AF.Sqrt,
                                     bias=eps_t[0:1], scale=1.0)
                rstd = l2s.tile([1, 512], F32R, tag="rstd2")
                with nc.allow_low_precision(reason="rstd rounded to f32r"):
                    nc.vector.reciprocal(out=rstd, in_=sd)
                nmr = l2s.tile([1, 512], F32R, tag="nmr2")
                nc.vector.scalar_tensor_tensor(
                    out=nmr, in0=mu, scalar=-1.0, in1=rstd,
                    op0=ALU.mult, op1=ALU.mult)
                # broadcast rstd / (-mu*rstd) across partitions via PE
                bc_r = bcp.tile([128, 512], F32, tag="bc_r")
                nc.tensor.matmul(bc_r, ones_row, rstd,
                                 start=True, stop=True)
                bc_m = bcp.tile([128, 512], F32, tag="bc_m")
                nc.tensor.matmul(bc_m, ones_row, nmr,
                                 start=True, stop=True)
                for ct in range(CT):
                    t = sqw.tile([128, 512], F32, tag="h2tmp")
                    nc.vector.tensor_mul(t, x2T[ct][:, cs], bc_r)
                    nc.vector.tensor_add(t, t, bc_m)
                    nc.scalar.activation(out=h2T[ch][ct], in_=t, func=AF.Identity,
                                         scale=g2_s[:, ct:ct + 1],
                                         bias=b2_s[:, ct:ct + 1])

            # MLP, hid-split in halves so fc1/fc2 weights load exactly once.
            # Output accumulates across the two halves in SBUF.
            HH = HT // 2  # 12 hid tiles per half
            with (
                tc.tile_pool(name="g_sb", bufs=1) as g_pool,
                tc.tile_pool(name="acc_sb", bufs=1) as acc_pool,
                tc.tile_pool(name="out_sb", bufs=3) as osb,
                tc.tile_pool(name="f_psum", bufs=2, space="PSUM") as fps,
            ):
                g_sb = [g_pool.tile([128, 512], F32R, tag=f"g{ht}",
                                    name=f"g{ht}") for ht in range(HH)]
                acc = [[acc_pool.tile([128, 512], F32, tag=f"acc{ft}_{ch}",
                                      name=f"acc{ft}_{ch}")
                        for ch in range(QCH)] for ft in range(CT)]
                for hb in range(2):
                    with (
                        tc.tile_pool(name="wfc1", bufs=1) as w1_pool,
                        tc.tile_pool(name="wfc2", bufs=1) as w2_pool,
                    ):
                        w1 = w1_pool.tile([128, CT, HID // 2], F32R, tag="wfc1")
                        dma(out=w1,
                            in_=wfc1T.rearrange("(ct p) f -> p ct f", p=128)
                            [:, :, hb * (HID // 2):(hb + 1) * (HID // 2)])
                        w2 = w2_pool.tile([128, HH, C], F32R, tag="wfc2")
                        dma(out=w2,
                            in_=wfc2T.rearrange("(ht p) f -> p ht f", p=128)
                            [:, hb * HH:(hb + 1) * HH, :])
                        for ch in range(QCH):
                            cs = slice(ch * 512, (ch + 1) * 512)
                            for ht in range(HH):
                                ps = fps.tile([128, 512], F32, tag="f1")
                                for ct in range(CT):
                                    nc.tensor.matmul(
                                        ps, w1[:, ct, ht * 128:(ht + 1) * 128],
                                        h2T[ch][ct],
                                        start=(ct == 0), stop=(ct == CT - 1))
                                nc.scalar.activation(
                                    out=g_sb[ht], in_=ps, func=AF.Gelu,
                                    bias=f1b_s[:, hb * HH + ht:hb * HH + ht + 1],
                                    scale=1.0)
                            for ft in range(CT):
                                ps = fps.tile([128, 512], F32, tag="f1")
                                for ht in range(HH):
                                    nc.tensor.matmul(
                                        ps, w2[:, ht, ft * 128:(ft + 1) * 128],
                                        g_sb[ht],
                                        start=(ht == 0), stop=(ht == HH - 1))
                                if hb == 0:
                                    # acc = psum + fc2_b + x2 (residual)
                                    nc.vector.scalar_tensor_tensor(
                                        out=acc[ft][ch], in0=ps,
                                        scalar=f2b_s[:, ft:ft + 1],
                                        in1=x2T[ft][:, cs],
                                        op0=ALU.add, op1=ALU.add)
                                else:
                                    ot = osb.tile([128, 512], F32, tag="ot")
                                    nc.vector.tensor_add(ot, ps, acc[ft][ch])
                                    dma(out=outT[ft * 128:(ft + 1) * 128, cs],
                                        in_=ot)


def kernel(**inputs):
    global _NC_CACHE, LAST_RESULTS
    import os
    ins = {k: np.ascontiguousarray(np.asarray(v, dtype=np.float32))
           for k, v in inputs.items()}
    if _NC_CACHE is None:
        _NC_CACHE = build_program()
    nc = _NC_CACHE

    shared = {
        "wqkvT": np.ascontiguousarray(ins["qkv_w"].T),
        "wprojT": np.ascontiguousarray(ins["proj_w"].T),
        "wfc1T": np.ascontiguousarray(ins["fc1_w"].T),
        "wfc2T": np.ascontiguousarray(ins["fc2_w"].T),
        "pb": ins["proj_b"], "f1b": ins["fc1_b"], "f2b": ins["fc2_b"],
        "g1": ins["ln1_g"], "b1": ins["ln1_b"],
        "g2": ins["ln2_g"], "b2": ins["ln2_b"],
    }
    in_maps = []
    for s in range(NCORES):
        b, half = s // 2, s % 2
        m = dict(shared)
        m["xb"] = np.ascontiguousarray(np.roll(ins["x"][b], -half * NO, axis=0))
        in_maps.append(m)

    trace = bool(int(os.environ.get("KBENCH_TRACE", "0")))
    LAST_RESULTS = run_bass_kernel_spmd(
        nc, in_maps, core_ids=list(range(NCORES)), trace=trace)
    out = np.empty((B, N, C), np.float32)
    for s in range(NCORES):
        b, half = s // 2, s % 2
        out[b, half * NO:(half + 1) * NO, :] = LAST_RESULTS.results[s]["outT"].T
    return out
